# revision 22
# baseline (speedup 1.0000x reference)
"""GCN2Conv (variant=False) Trainium2 kernel, v11.

Math (all linear, so theta folds out of the critical path):
  out = support @ T',              T' = beta*theta + (1-beta)*I
  support = c1*hi + alpha*h0,      c1 = 1-alpha
  hi = dis_r . ((A+I) @ (dis . x)),  dis = (rowsum(A)+1)^-1/2
=>
  out[R] = dis[R] . (A_R @ Gd) + dis[R]^2 . G[R] + H[R]
  G = c1 * (x @ T'),  Gd = dis . G,  H = alpha * (h0[R] @ T')

Sharding: B=4 graphs x 2 cores/graph. Core pair (2g, 2g+1) owns rows
[0:1536) / [1536:3000) of graph g (128-aligned halves; everything zero
padded to N_PAD=3072, M_PAD=1536 so the SPMD program is identical on
both halves, and AllGather outputs land on 128-aligned global node
ranges for both ranks).

Host passes pure layout transforms only (slice / transpose / pad / fp16
cast): AT = A[R,:].T as [nch, 128, KT*chunk] fp16 so each stream DMA
reads per-partition-contiguous memory; x_full pre-arranged [128, KT*F];
x[R].T / h0[R].T / theta packed into one fp16 tensor. Output is
produced transposed [F, M_PAD] fp32; the host transposes back.

Device pipeline per core:
  - 6 stream DMAs (3 m-chunks x 2 halves) of A^T into SBUF.
  - Row degrees via 4x column-tiled PE ones-matmuls chasing the stream.
  - 3 pipelined pair-AllGathers (1KB fp16 each) exchange degree chunks;
    AG_c unlocks "wave" c = 8 k-blocks (4 per rank side).
  - Main matmul rawT[f,m] += Gd_kb^T AT_kb accumulates 24 k-blocks into
    3 PSUM banks; waves 0/1 run while later chunks stream, wave 2 plus
    per-chunk epilogue/store is the only post-AG_2 tail, preceded by a
    dep-anchored PE warmup burst (HAM clock).
"""

import math
import sys

import numpy as np

sys.path.insert(0, "/opt/trn_rl_repo")

import concourse.bacc as bacc
import concourse.mybir as mybir
import concourse.tile as tile
from concourse import bass_utils, masks
from concourse.mybir import dt
from concourse.tile import add_dep_helper as _adh

AF = mybir.ActivationFunctionType

F = 128            # feature dim
P = 128            # SBUF partitions

B_FULL, N_FULL = 4, 3000
N_CORES_FULL = 8
M_PAD_FULL = 1536          # even core rows [0:1536), odd [1536:3000)
N_PAD_FULL = 3072
CHUNK_FULL = 512
NCH = 3                    # m-chunks / AG waves (schedule hardcoded for 3)
N_WARM = 6                 # dep-anchored PE re-warm matmuls before wave 2


def build_program(n_pad, m_pad, chunk, n_cores, alpha, beta, n_quarters=4):
    """Build the SPMD Bass program (identical on every core)."""
    assert n_pad == 2 * m_pad and m_pad % chunk == 0 and chunk % P == 0
    KT = n_pad // P                 # k tiles (contraction blocks)
    nch = m_pad // chunk            # m-chunks == AG waves
    assert nch == NCH
    K = chunk // P                  # k-blocks unlocked per wave per side
    kb_odd = m_pad // P             # first odd-rank k-block
    assert KT % n_quarters == 0
    kb_per_q = KT // n_quarters
    c1 = 1.0 - alpha

    # wave_c k-blocks: even-rank rows [c*K,(c+1)*K) + odd-rank shifted
    waves = [list(range(c * K, (c + 1) * K))
             + list(range(kb_odd + c * K, kb_odd + (c + 1) * K))
             for c in range(nch)]

    nc = bacc.Bacc(
        "TRN2", target_bir_lowering=False, debug=False, num_devices=n_cores
    )
    adjT = nc.dram_tensor(
        "adjT", [nch * P * KT, chunk], dt.float16, kind="ExternalInput"
    )
    x_full = nc.dram_tensor("x_full", [P, KT * F], dt.float16,
                            kind="ExternalInput")
    xmisc_d = nc.dram_tensor("xmisc", [F, 2 * m_pad + F + 8], dt.float16,
                             kind="ExternalInput")
    outT_d = nc.dram_tensor("outT", [F, m_pad], dt.float32, kind="ExternalOutput")

    groups = [[2 * g, 2 * g + 1] for g in range(max(1, n_cores // 2))]

    with tile.TileContext(nc) as tc:
        from contextlib import ExitStack

        with ExitStack() as ctx:
            ep = ctx.enter_context

            consts = ep(tc.tile_pool(name="consts", bufs=1))
            at_pool = ep(tc.tile_pool(name="at", bufs=1))
            xs_pool = ep(tc.tile_pool(name="xs", bufs=1))
            deg_pool = ep(tc.tile_pool(name="deg", bufs=1))
            out_pool = ep(tc.tile_pool(name="out", bufs=1))
            ps_raw = ep(tc.tile_pool(name="ps_raw", bufs=3, space="PSUM"))
            ps_deg = ep(tc.tile_pool(name="ps_deg", bufs=2, space="PSUM"))
            ps_sm = ep(tc.tile_pool(name="ps_sm", bufs=2, space="PSUM"))
            ps_spin = ep(tc.tile_pool(name="ps_spin", bufs=1, space="PSUM"))
            dram = ep(tc.tile_pool(name="dram", bufs=1, space="DRAM"))

            # ---------------- constants + small loads ----------------------
            ident = consts.tile([P, P], dt.float32)
            masks.make_identity(nc, ident[:])
            ident16 = consts.tile([P, P], dt.float16)
            nc.vector.tensor_copy(ident16[:], ident[:])
            ones = consts.tile([P, 1], dt.float16)
            nc.gpsimd.memset(ones[:], 1.0)

            xg = xs_pool.tile([P, KT * F], dt.float16)
            nc.gpsimd.dma_start(xg[:], x_full[:])
            xmisc = xs_pool.tile([F, 2 * m_pad + F + 8], dt.float16, tag="xmisc")
            nc.gpsimd.dma_start(xmisc[:], xmisc_d[:])
            xTl = xmisc[:, 0:m_pad]
            h0T_sb = xmisc[:, m_pad : 2 * m_pad]
            theta16 = xmisc[:, 2 * m_pad : 2 * m_pad + F]

            # T' = beta*theta + (1-beta)*I ; thG = c1*T' ; thH = alpha*T'
            thetaP = consts.tile([F, F], dt.float32)
            nc.vector.tensor_scalar_mul(thetaP[:], theta16, beta)
            nc.vector.scalar_tensor_tensor(
                thetaP[:], ident[:], 1.0 - beta, thetaP[:],
                mybir.AluOpType.mult, mybir.AluOpType.add,
            )
            thG = consts.tile([F, F], dt.float16)
            nc.vector.tensor_scalar_mul(thG[:], thetaP[:], c1)
            thH = consts.tile([F, F], dt.float16)
            nc.vector.tensor_scalar_mul(thH[:], thetaP[:], alpha)

            # ---------------- A^T stream ------------------------------------
            AT = at_pool.tile([P, nch * KT * chunk], dt.float16)
            AT4 = AT[:].rearrange("p (c kb m) -> p c kb m", c=nch, kb=KT)
            AT2 = AT[:]
            adjT_flat = adjT[:].rearrange("(c p kb) m -> p c (kb m)", c=nch, p=P)

            def emit_stream(c, h=None):
                run = KT * chunk
                if h is None:
                    nc.sync.dma_start(
                        AT2[:, c * run : (c + 1) * run],
                        adjT_flat[:, c, :],
                    )
                else:
                    half = run // 2
                    nc.sync.dma_start(
                        AT2[:, c * run + h * half : c * run + (h + 1) * half],
                        adjT_flat[:, c, h * half : (h + 1) * half],
                    )

            emit_stream(0, 0)
            emit_stream(0, 1)
            emit_stream(1)
            emit_stream(2)

            # ---------------- degree rowsums (4x col-tiled PE) --------------
            degrow = deg_pool.tile([1, m_pad], dt.float32)
            deg16 = deg_pool.tile([1, m_pad], dt.float16, tag="deg16")
            rcp = deg_pool.tile([1, m_pad], dt.float32, tag="rcp")
            rs_row = deg_pool.tile([1, m_pad], dt.float32, tag="rs_row")
            rs_b = deg_pool.tile([P, m_pad], dt.float32, tag="rs_b")
            disg = deg_pool.tile([P, KT], dt.float32, tag="disg")
            deg_ps_tiles = {}

            def emit_rowsums(c, q):
                if q == 0:
                    deg_ps_tiles[c] = ps_deg.tile(
                        [P, chunk], dt.float32, name=f"deg_ps_{c}",
                        tag="degps", bufs=2,
                    )
                dps = deg_ps_tiles[c]
                for kb in range(q * kb_per_q, (q + 1) * kb_per_q):
                    j = kb % 4
                    nc.tensor.matmul(
                        dps[32 * j : 32 * j + 1, :], ones[:, 0:1],
                        AT4[:, c, kb, :],
                        start=(kb < 4), stop=(kb >= KT - 4),
                        tile_position=(0, 32 * j),
                    )

            def emit_deg_chunk_post(c):
                s = c * chunk
                dps = deg_ps_tiles[c]
                t4a = deg_pool.tile([1, chunk], dt.float32, tag="t4a")
                nc.vector.tensor_copy(t4a[0:1, :], dps[0:1, :])
                for r in (32, 64, 96):
                    nc.vector.tensor_add(
                        t4a[0:1, :], t4a[0:1, :], dps[r : r + 1, :]
                    )
                nc.vector.tensor_scalar_add(
                    degrow[0:1, s : s + chunk], t4a[0:1, :], 1.0
                )
                nc.vector.tensor_copy(
                    deg16[0:1, s : s + chunk], degrow[0:1, s : s + chunk]
                )
                nc.vector.reciprocal(
                    rcp[0:1, s : s + chunk], degrow[0:1, s : s + chunk]
                )
                nc.scalar.sqrt(
                    rs_row[0:1, s : s + chunk], rcp[0:1, s : s + chunk]
                )

            def emit_rsb(c):
                s = c * chunk
                nc.gpsimd.partition_broadcast(
                    rs_b[:, s : s + chunk], rs_row[0:1, s : s + chunk]
                )

            # ---------------- x^T transposes + G ----------------------------
            xT = xs_pool.tile([P, KT * F], dt.float16, tag="xT")
            G = xs_pool.tile([P, KT * F], dt.float16, tag="G")
            Gd = xs_pool.tile([P, KT * F], dt.float16, tag="Gd")

            def emit_xt(kb):
                tp = ps_sm.tile([P, P], dt.float16, tag="sm")
                nc.tensor.transpose(
                    tp[:P, :P], xg[:, kb * F : (kb + 1) * F], ident16[:P, :P]
                )
                nc.scalar.activation(
                    xT[:, kb * F : (kb + 1) * F], tp[:P, :P], AF.Copy
                )

            def emit_g(kb):
                gp = ps_sm.tile([P, F], dt.float32, tag="sm")
                nc.tensor.matmul(
                    gp[:P, :F], xT[:, kb * F : (kb + 1) * F], thG[:, :],
                    start=True, stop=True,
                )
                nc.scalar.activation(
                    G[:, kb * F : (kb + 1) * F], gp[:P, :F], AF.Copy
                )

            # ---------------- QT = (G_R / deg_R + H)^T ----------------------
            QT = out_pool.tile([P, m_pad], dt.float32, tag="QT")
            GoT = out_pool.tile([P, m_pad], dt.float16, tag="GoT")
            rs2_b = deg_pool.tile([P, chunk], dt.float32, tag="rs2_b")

            def emit_goh(c):
                s = c * chunk
                hp = ps_sm.tile([P, chunk], dt.float32, tag="sm")
                nc.tensor.matmul(
                    hp[:F, :chunk], thH[:, :], h0T_sb[:, s : s + chunk],
                    start=True, stop=True,
                )
                nc.scalar.activation(QT[:, s : s + chunk], hp[:F, :chunk], AF.Copy)
                gp2 = ps_sm.tile([P, chunk], dt.float32, tag="sm")
                nc.tensor.matmul(
                    gp2[:F, :chunk], thG[:, :], xTl[:, s : s + chunk],
                    start=True, stop=True,
                )
                nc.scalar.activation(GoT[:, s : s + chunk], gp2[:F, :chunk], AF.Copy)

            def emit_qt(c):
                s = c * chunk
                nc.gpsimd.partition_broadcast(rs2_b[:, :], rcp[0:1, s : s + chunk])
                tmp = deg_pool.tile([P, chunk], dt.float32, tag="qtmp")
                nc.vector.tensor_mul(tmp[:, :], GoT[:, s : s + chunk], rs2_b[:, :])
                nc.vector.tensor_add(
                    QT[:, s : s + chunk], QT[:, s : s + chunk], tmp[:, :]
                )

            # ---------------- pipelined degree AllGathers -------------------
            deg_loc_d = dram.tile([m_pad], dt.float16, name="deg_loc_d")
            deg_pair_d = [dram.tile([2 * chunk], dt.float16, tag=f"dp{c}",
                                    name=f"deg_pair_{c}")
                          for c in range(nch)]
            dg_loads = {}

            def emit_degout(c):
                s = c * chunk
                nc.gpsimd.dma_start(
                    deg_loc_d[s : s + chunk].rearrange("(a m) -> a m", a=1),
                    deg16[0:1, s : s + chunk],
                )

            def emit_agop(c):
                s = c * chunk
                nc.gpsimd.collective_compute(
                    "AllGather",
                    mybir.AluOpType.bypass,
                    replica_groups=groups,
                    ins=[deg_loc_d[s : s + chunk]],
                    outs=[deg_pair_d[c][:]],
                )

            def emit_wave_dis(c):
                for side in range(2):
                    dg = deg_pool.tile([K, P], dt.float16, tag="dgT", bufs=2)
                    ld = nc.sync.dma_start(
                        dg[:, :],
                        deg_pair_d[c][side * chunk : (side + 1) * chunk]
                        .rearrange("(a b) -> a b", b=P),
                    )
                    dg_loads[(c, side)] = ld
                    tp = ps_sm.tile([P, K], dt.float16, tag="sm")
                    nc.tensor.transpose(tp[:P, :K], dg[:K, :P], ident16[:K, :K])
                    kb0 = side * kb_odd + c * K
                    nc.vector.reciprocal(disg[:, kb0 : kb0 + K], tp[:P, :K])
                    nc.scalar.sqrt(disg[:, kb0 : kb0 + K], disg[:, kb0 : kb0 + K])

            def emit_gd(kbs):
                for kb in kbs:
                    nc.vector.tensor_scalar_mul(
                        Gd[:, kb * F : (kb + 1) * F],
                        G[:, kb * F : (kb + 1) * F],
                        disg[:, kb : kb + 1],
                    )

            # ---------------- main matmul + epilogue ------------------------
            raw_ps = [ps_raw.tile([P, chunk], dt.float32, name=f"raw_{c}",
                                  tag=f"raw{c}", bufs=1)
                      for c in range(nch)]
            n_mm_done = [0] * nch
            outT_sb = out_pool.tile([P, m_pad], dt.float32, tag="outT")

            def emit_mm(kbs, c):
                for kb in kbs:
                    nc.tensor.matmul(
                        raw_ps[c][:F, :chunk],
                        Gd[:, kb * F : (kb + 1) * F],
                        AT4[:, c, kb, :],
                        start=(n_mm_done[c] == 0),
                        stop=(n_mm_done[c] == KT - 1),
                    )
                    n_mm_done[c] += 1

            def emit_warm(n, anchor):
                # scratch matmuls dep-anchored on the last AllGather's output
                # load: they run cold right as the exchange lands and pull
                # the PE clock back to 2.4GHz for the wave-2 tail
                sp = ps_spin.tile([P, chunk], dt.float32, tag="spin", bufs=1)
                for _ in range(n):
                    wmm = nc.tensor.matmul(
                        sp[:F, :chunk], thG[:, :], AT4[:, 0, 0, :],
                        start=True, stop=True, skip_group_check=True,
                    )
                    _adh(wmm.ins, anchor.ins, sync=True,
                         reason="warmup anchored after last AG load")

            def emit_epilogue(c):
                s = c * chunk
                nc.vector.tensor_mul(
                    outT_sb[:, s : s + chunk], raw_ps[c][:F, :chunk],
                    rs_b[:, s : s + chunk],
                )
                nc.vector.tensor_add(
                    outT_sb[:, s : s + chunk], outT_sb[:, s : s + chunk],
                    QT[:, s : s + chunk],
                )
                nc.sync.dma_start(
                    outT_d[:, s : s + chunk], outT_sb[:, s : s + chunk]
                )

            # ---------------- emission schedule (engines in-order) ----------
            # The degree/AG chain leads everything: rowsums are the PE's
            # first work, their posts the DVE's first work after theta, and
            # the deg-out DMAs ride the otherwise-idle SWDGE path, so each
            # AllGather fires as early as the collective stream allows.
            for c in range(nch):
                for q in range(n_quarters):
                    emit_rowsums(c, q)
                emit_deg_chunk_post(c)
                emit_degout(c)
            for c in range(nch):
                emit_agop(c)
            for kb in range(KT):
                emit_xt(kb)
            for kb in range(KT):
                emit_g(kb)
            for c in range(nch):
                emit_goh(c)
            for c in range(nch):
                emit_rsb(c)
            for c in range(nch):
                emit_qt(c)
            emit_wave_dis(0)
            emit_warm(N_WARM, dg_loads[(0, 0)])
            emit_gd(waves[0])
            emit_mm(waves[0], 0)
            emit_mm(waves[0], 1)
            emit_mm(waves[0], 2)
            emit_wave_dis(1)
            emit_gd(waves[1])
            emit_mm(waves[1], 0)
            emit_mm(waves[1], 1)
            emit_mm(waves[1], 2)
            emit_wave_dis(2)
            emit_gd(waves[2])
            emit_mm(waves[2], 0)
            emit_epilogue(0)
            emit_mm(waves[2], 1)
            emit_epilogue(1)
            emit_mm(waves[2], 2)
            emit_epilogue(2)

    nc.compile()
    return nc


def make_in_maps(x, adj, h0, theta, n_cores, n_pad, m_pad, chunk, n_real):
    KT = n_pad // P
    nch = m_pad // chunk
    f2 = np.float16
    in_maps = []
    x_gs = {}
    for c in range(n_cores):
        g, h = c // 2, c % 2
        r0 = 0 if h == 0 else m_pad
        m_real = m_pad if h == 0 else n_real - m_pad
        if g not in x_gs:
            xp = np.zeros((n_pad, F), f2)
            xp[:n_real] = x[g].astype(f2)
            x_gs[g] = np.ascontiguousarray(
                xp.reshape(KT, P, F).transpose(1, 0, 2)
            ).reshape(P, KT * F)
        at = np.zeros((n_pad, m_pad), f2)
        at[:n_real, :m_real] = adj[g, r0 : r0 + m_real, :].astype(f2).T
        at = np.ascontiguousarray(
            at.reshape(KT, P, nch, chunk).transpose(2, 1, 0, 3)
        ).reshape(nch * P * KT, chunk)
        xmisc = np.zeros((F, 2 * m_pad + F + 8), f2)
        xmisc[:, :m_real] = x[g, r0 : r0 + m_real, :].astype(f2).T
        xmisc[:, m_pad : m_pad + m_real] = h0[g, r0 : r0 + m_real, :].astype(f2).T
        xmisc[:, 2 * m_pad : 2 * m_pad + F] = theta.astype(f2)
        in_maps.append(
            {
                "adjT": at,
                "x_full": x_gs[g],
                "xmisc": xmisc,
            }
        )
    return in_maps


_CACHE = {}


def _get_program(key, *args, **kwargs):
    if key not in _CACHE:
        _CACHE[key] = build_program(*args, **kwargs)
    return _CACHE[key]


def kernel(x, adj, h0, theta, lamda, alpha, l):
    x = np.asarray(x, dtype=np.float32)
    adj = np.asarray(adj, dtype=np.float32)
    h0 = np.asarray(h0, dtype=np.float32)
    theta = np.asarray(theta, dtype=np.float32)
    lamda_f = float(np.asarray(lamda))
    alpha_f = float(np.asarray(alpha))
    l_f = float(np.asarray(l))
    beta_f = float(math.log(lamda_f / l_f + 1.0))

    B, N, Fdim = x.shape
    assert (B, N, Fdim) == (B_FULL, N_FULL, F)

    nc = _get_program(
        ("full", alpha_f, beta_f),
        N_PAD_FULL, M_PAD_FULL, CHUNK_FULL, N_CORES_FULL, alpha_f, beta_f,
    )

    in_maps = make_in_maps(
        x, adj, h0, theta, N_CORES_FULL,
        N_PAD_FULL, M_PAD_FULL, CHUNK_FULL, N_FULL,
    )
    res = bass_utils.run_bass_kernel_spmd(
        nc, in_maps, list(range(N_CORES_FULL))
    ).results

    out = np.empty((B, N, Fdim), dtype=np.float32)
    for c in range(N_CORES_FULL):
        g, h = c // 2, c % 2
        r0 = 0 if h == 0 else M_PAD_FULL
        m_real = M_PAD_FULL if h == 0 else N - M_PAD_FULL
        out[g, r0 : r0 + m_real, :] = res[c]["outT"][:, :m_real].T
    return out
